# revision 1
# baseline (speedup 1.0000x reference)
"""Trainium2 Bass kernel for the MoE feed-forward block (nn_MoEFF).

Strategy: pure data-parallel over tokens. The 4096 tokens are split into
8 slices of 512; each NeuronCore runs the *entire* network on its slice
(router + all 8 experts dense-masked + shared expert). With E=8/K=4 every
expert serves ~half the tokens anyway, so dense-masked expert compute
costs only 2x the ideal sparse flops and avoids all collectives.

Layout: activations are kept transposed in SBUF ([feature-on-partition,
token-on-free]) so weight matrices in natural [in, out] layout are the
stationary matmul operand. All matmuls run in bf16 with fp32 PSUM
accumulation (measured end-to-end rel err ~2e-3 vs the fp32 reference);
all element-wise math stays fp32.
"""

from contextlib import ExitStack

import ml_dtypes
import numpy as np

B, S, D = 2, 2048, 1024
E, TOPK, H = 8, 4, 1024
SH = 2 * H
NCORES = 8
T = B * S                 # 4096 tokens
TPC = T // NCORES         # 512 tokens per core
KT = D // 128             # 8 contraction tiles
MT_H = H // 128           # 8
MT_SH = SH // 128         # 16
NT = TPC // 128           # 4 token sub-tiles (router)

bf16 = ml_dtypes.bfloat16

# Unit layout of the stacked weight tensor (each unit is a [1024, 1024]
# block of rows in `wall`): see _pack_weights.
U_LIN0 = 0
U_S1A = 1
U_S1B = 2
U_EXP = 3                 # 3 + 3*e + {0: w1, 1: w3, 2: w2}
U_SH1 = 27                # 27, 28: sh_w1 cols [0:1024], [1024:2048]
U_SH3 = 29                # 29, 30
U_SH2 = 31                # 31, 32: sh_w2 rows [0:1024], [1024:2048]
U_LIN1 = 33
U_S2A = 34
U_S2B = 35
N_UNITS = 36

_prog = None  # built once per process
last_results = None  # BassKernelResults of the most recent kernel() call


def _build_program():
    import concourse.bacc as bacc
    import concourse.mybir as mybir
    import concourse.tile as tile

    F32, BF = mybir.dt.float32, mybir.dt.bfloat16
    AF = mybir.ActivationFunctionType
    OP = mybir.AluOpType

    nc = bacc.Bacc()

    wall_d = nc.dram_tensor("wall", [N_UNITS * 1024, 1024], BF, kind="ExternalInput")
    xT_d = nc.dram_tensor("xT", [D, TPC], BF, kind="ExternalInput")
    gate_d = nc.dram_tensor("gateT", [D, E], BF, kind="ExternalInput")
    bias_d = nc.dram_tensor("biases", [128, 6 * KT], mybir.dt.float32, kind="ExternalInput")
    ident_d = nc.dram_tensor("ident", [128, 128], mybir.dt.float32, kind="ExternalInput")
    sel_d = nc.dram_tensor("sel", [E, E * 128], mybir.dt.float32, kind="ExternalInput")
    out_d = nc.dram_tensor("outT", [D, TPC], mybir.dt.float32, kind="ExternalOutput")

    with tile.TileContext(nc) as tc, ExitStack() as ctx:
        wp = ctx.enter_context(tc.tile_pool(name="wp", bufs=5))
        sp = ctx.enter_context(tc.tile_pool(name="sp", bufs=1))
        dp = ctx.enter_context(tc.tile_pool(name="dp", bufs=4))
        pp = ctx.enter_context(tc.tile_pool(name="pp", bufs=2, space="PSUM"))

        def wload(unit):
            wt = wp.tile([128, KT, 1024], BF, tag="wmat", name=f"w{unit}")
            nc.sync.dma_start(
                wt[:],
                wall_d[unit * 1024:(unit + 1) * 1024, :].rearrange(
                    "(k p) c -> p k c", p=128
                ),
            )
            return wt

        # ---- static inputs ----
        xbf = sp.tile([128, KT, TPC], BF, tag="xbf", name="xbf")
        nc.sync.dma_start(xbf[:], xT_d[:].rearrange("(k p) t -> p k t", p=128))
        gsb = sp.tile([128, KT, E], BF, tag="gsb", name="gsb")
        nc.sync.dma_start(gsb[:], gate_d[:].rearrange("(k p) e -> p k e", p=128))
        biases = sp.tile([128, 6 * KT], F32, tag="biases", name="biases")
        nc.sync.dma_start(biases[:], bias_d[:])
        ident = sp.tile([128, 128], F32, tag="ident", name="ident")
        nc.sync.dma_start(ident[:], ident_d[:])
        sel = sp.tile([E, E * 128], F32, tag="sel", name="sel")
        nc.sync.dma_start(sel[:], sel_d[:])

        def bcol(idx, n):
            # per-partition bias column n of bias group idx
            return biases[:, idx * KT + n:idx * KT + n + 1]

        # ---- block 1: h0 = x @ lin0 + b ----
        w_lin0 = wload(U_LIN0)
        h0 = []
        for n in range(KT):
            ps = pp.tile([128, TPC], F32, tag="gu", bufs=4, name="ps_h0")
            for k in range(KT):
                nc.tensor.matmul(ps[:], w_lin0[:, k, n * 128:(n + 1) * 128],
                                 xbf[:, k, :], start=(k == 0), stop=(k == KT - 1))
            t = sp.tile([128, TPC], BF, tag="h0", bufs=8, name=f"h0_{n}")
            nc.scalar.activation(t[:], ps[:], AF.Identity, bias=bcol(0, n))
            h0.append(t)

        # ---- swiglu 1 -> h (the MoE input) ----
        w_s1a, w_s1b = wload(U_S1A), wload(U_S1B)
        hbf = []
        for m in range(KT):
            pa = pp.tile([128, TPC], F32, tag="gu", bufs=4, name="ps_a1")
            for k in range(KT):
                nc.tensor.matmul(pa[:], w_s1a[:, k, m * 128:(m + 1) * 128],
                                 h0[k][:], start=(k == 0), stop=(k == KT - 1))
            pb = pp.tile([128, TPC], F32, tag="gu", bufs=4, name="ps_b1")
            for k in range(KT):
                nc.tensor.matmul(pb[:], w_s1b[:, k, m * 128:(m + 1) * 128],
                                 h0[k][:], start=(k == 0), stop=(k == KT - 1))
            sg = dp.tile([128, TPC], F32, tag="gs", name="sg1")
            nc.scalar.activation(sg[:], pa[:], AF.Sigmoid, bias=bcol(1, m))
            sa = dp.tile([128, TPC], F32, tag="v", name="sa1")
            nc.vector.scalar_tensor_tensor(sa[:], pa[:], bcol(1, m), sg[:],
                                           OP.add, OP.mult)
            t = sp.tile([128, TPC], BF, tag="hbf", bufs=8, name=f"hbf_{m}")
            nc.vector.scalar_tensor_tensor(t[:], pb[:], bcol(2, m), sa[:],
                                           OP.add, OP.mult)
            hbf.append(t)

        # ---- router matmuls: z[t, e] for all 4 token sub-tiles ----
        z_all = pp.tile([128, NT * E], F32, tag="misc", bufs=2, name="z_all")
        for t in range(NT):
            for k in range(KT):
                nc.tensor.matmul(z_all[:, t * E:(t + 1) * E],
                                 hbf[k][:, t * 128:(t + 1) * 128],
                                 gsb[:, k, :], start=(k == 0), stop=(k == KT - 1))

        # ---- router chain (DVE/ACT; overlaps expert-0 g/u matmuls on PE) ----
        ez = sp.tile([128, NT * E], F32, tag="ez", name="ez")
        cur = sp.tile([128, NT * E], F32, tag="cur", name="cur")
        cm = sp.tile([128, NT * E], F32, tag="cm", name="cm")
        combine = sp.tile([128, NT * E], F32, tag="combine", name="combine")
        stat = sp.tile([128, 4 * NT], F32, tag="stat", name="stat")  # nmx, thr, s, r

        for t in range(NT):
            zt = z_all[:, t * E:(t + 1) * E]
            nmx = stat[:, t:t + 1]
            nc.vector.tensor_reduce(nmx, zt, mybir.AxisListType.X, OP.max, negate=True)
            ezt = ez[:, t * E:(t + 1) * E]
            nc.scalar.activation(ezt, zt, AF.Exp, bias=nmx, scale=1.0)
            curt = cur[:, t * E:(t + 1) * E]
            nc.vector.tensor_copy(curt, ezt)
            thr = stat[:, NT + t:NT + t + 1]
            for i in range(TOPK):
                nc.vector.tensor_reduce(thr, curt, mybir.AxisListType.X, OP.max)
                if i < TOPK - 1:
                    eq = dp.tile([128, E], F32, tag="eq", bufs=2, name="eq")
                    nc.vector.tensor_scalar(eq[:], curt, thr, None, OP.is_equal)
                    nc.vector.scalar_tensor_tensor(curt, eq[:], -1e30, curt,
                                                   OP.mult, OP.add)
            cmt = cm[:, t * E:(t + 1) * E]
            # cm = ez * (ez >= thr); reuse cur as the mask buffer
            nc.vector.tensor_scalar(curt, ezt, thr, None, OP.is_ge)
            nc.vector.tensor_mul(cmt, ezt, curt)
            s = stat[:, 2 * NT + t:2 * NT + t + 1]
            nc.vector.tensor_reduce(s, cmt, mybir.AxisListType.X, OP.add)
            r = stat[:, 3 * NT + t:3 * NT + t + 1]
            nc.vector.reciprocal(r, s)
            nc.vector.tensor_scalar(combine[:, t * E:(t + 1) * E], cmt, r, None,
                                    OP.mult)

        cbT = sp.tile([E, TPC], F32, tag="cbT", name="cbT")

        def emit_transposes():
            for t in range(NT):
                trp = pp.tile([E, 128], F32, tag="misc", name="trp")
                nc.tensor.transpose(trp[:], combine[:, t * E:(t + 1) * E], ident[:])
                nc.scalar.activation(cbT[0:E, t * 128:(t + 1) * 128], trp[:], AF.Copy)

        def outer(e):
            # cb_ps[p, t] = sum_k sel[k, e*128+p] * cbT[k, t] = combine[t, e]
            cb_ps = pp.tile([128, TPC], F32, tag="misc", name="cb_ps")
            nc.tensor.matmul(cb_ps[:], sel[:, e * 128:(e + 1) * 128], cbT[0:E, :],
                             start=True, stop=True)
            return cb_ps

        def emit_gu(w1, w3, n_m, cb_ps):
            """g/u/v for one expert (n_m m-tiles). Returns vb tiles (bf16),
            scaled by cb_ps when given. w1/w3 are lists of wmat units."""
            vbs = []
            for m in range(n_m):
                u, mm = divmod(m, KT)
                pg = pp.tile([128, TPC], F32, tag="gu", bufs=4, name="ps_g")
                for k in range(KT):
                    nc.tensor.matmul(pg[:], w1[u][:, k, mm * 128:(mm + 1) * 128],
                                     hbf[k][:], start=(k == 0), stop=(k == KT - 1))
                pu = pp.tile([128, TPC], F32, tag="gu", bufs=4, name="ps_u")
                for k in range(KT):
                    nc.tensor.matmul(pu[:], w3[u][:, k, mm * 128:(mm + 1) * 128],
                                     hbf[k][:], start=(k == 0), stop=(k == KT - 1))
                sg = dp.tile([128, TPC], F32, tag="gs", name="sg")
                nc.scalar.activation(sg[:], pg[:], AF.Sigmoid)
                gs = dp.tile([128, TPC], F32, tag="v", name="gs")
                nc.vector.tensor_mul(gs[:], pg[:], sg[:])
                vb = dp.tile([128, TPC], BF, tag="vb", bufs=24, name="vb")
                if cb_ps is None:
                    nc.vector.tensor_mul(vb[:], gs[:], pu[:])
                else:
                    v = dp.tile([128, TPC], F32, tag="v", name="v")
                    nc.vector.tensor_mul(v[:], gs[:], pu[:])
                    nc.vector.tensor_mul(vb[:], v[:], cb_ps[:])
                vbs.append(vb)
            return vbs

        def emit_y(w2, vbs, acc, cb_sb=None):
            """y = vb @ w2 accumulated into acc (fp32 SBUF). w2: list of units."""
            n_m = len(vbs)
            for n in range(KT):
                py = pp.tile([128, TPC], F32, tag="y", bufs=2, name="ps_y")
                for m in range(n_m):
                    u, mm = divmod(m, KT)
                    nc.tensor.matmul(py[:], w2[u][:, mm, n * 128:(n + 1) * 128],
                                     vbs[m][:], start=(m == 0), stop=(m == n_m - 1))
                if cb_sb is not None:
                    a = sp.tile([128, TPC], F32, tag="acc", bufs=8, name=f"acc_{n}")
                    nc.vector.tensor_mul(a[:], py[:], cb_sb[:])
                    acc.append(a)
                else:
                    nc.vector.tensor_add(acc[n][:], acc[n][:], py[:])

        # ---- experts ----
        acc = []
        for e in range(E):
            we1 = wload(U_EXP + 3 * e)
            we3 = wload(U_EXP + 3 * e + 1)
            we2 = wload(U_EXP + 3 * e + 2)
            if e == 0:
                vbs = emit_gu([we1], [we3], MT_H, None)
                emit_transposes()
                cb_ps0 = outer(0)
                cb_sb0 = dp.tile([128, TPC], F32, tag="cbsb", bufs=1, name="cb_sb0")
                nc.scalar.activation(cb_sb0[:], cb_ps0[:], AF.Copy)
                emit_y([we2], vbs, acc, cb_sb=cb_sb0)
            else:
                cb_ps = outer(e)
                vbs = emit_gu([we1], [we3], MT_H, cb_ps)
                emit_y([we2], vbs, acc)

        # ---- shared expert (always-on, unscaled) ----
        sh1 = [wload(U_SH1), wload(U_SH1 + 1)]
        sh3 = [wload(U_SH3), wload(U_SH3 + 1)]
        sh2 = [wload(U_SH2), wload(U_SH2 + 1)]
        vbs = emit_gu(sh1, sh3, MT_SH, None)
        emit_y(sh2, vbs, acc)

        # ---- block 3: lin1 + swiglu2 ----
        accbf = []
        for k in range(KT):
            t = sp.tile([128, TPC], BF, tag="h0", bufs=8, name=f"accbf_{k}")
            nc.scalar.activation(t[:], acc[k][:], AF.Copy)
            accbf.append(t)

        w_lin1 = wload(U_LIN1)
        h2 = []
        for n in range(KT):
            ps = pp.tile([128, TPC], F32, tag="gu", bufs=4, name="ps_h2")
            for k in range(KT):
                nc.tensor.matmul(ps[:], w_lin1[:, k, n * 128:(n + 1) * 128],
                                 accbf[k][:], start=(k == 0), stop=(k == KT - 1))
            t = sp.tile([128, TPC], BF, tag="hbf", bufs=8, name=f"h2_{n}")
            nc.scalar.activation(t[:], ps[:], AF.Identity, bias=bcol(3, n))
            h2.append(t)

        w_s2a, w_s2b = wload(U_S2A), wload(U_S2B)
        for m in range(KT):
            pa = pp.tile([128, TPC], F32, tag="gu", bufs=4, name="ps_a2")
            for k in range(KT):
                nc.tensor.matmul(pa[:], w_s2a[:, k, m * 128:(m + 1) * 128],
                                 h2[k][:], start=(k == 0), stop=(k == KT - 1))
            pb = pp.tile([128, TPC], F32, tag="gu", bufs=4, name="ps_b2")
            for k in range(KT):
                nc.tensor.matmul(pb[:], w_s2b[:, k, m * 128:(m + 1) * 128],
                                 h2[k][:], start=(k == 0), stop=(k == KT - 1))
            sg = dp.tile([128, TPC], F32, tag="gs", name="sg2")
            nc.scalar.activation(sg[:], pa[:], AF.Sigmoid, bias=bcol(4, m))
            sa = dp.tile([128, TPC], F32, tag="v", name="sa2")
            nc.vector.scalar_tensor_tensor(sa[:], pa[:], bcol(4, m), sg[:],
                                           OP.add, OP.mult)
            o = dp.tile([128, TPC], F32, tag="out", name="o")
            nc.vector.scalar_tensor_tensor(o[:], pb[:], bcol(5, m), sa[:],
                                           OP.add, OP.mult)
            nc.sync.dma_start(out_d[m * 128:(m + 1) * 128, :], o[:])

    # run_bass_via_pjrt serializes the BIR as-is; Bacc's lowering passes
    # (register allocation, TRN2 single-wait splitting) only run in
    # finalize(), so it must happen before dispatch.
    nc.finalize()
    return nc


def _pack_weights(inp):
    def b(a):
        return np.ascontiguousarray(np.asarray(a, dtype=np.float32)).astype(bf16)

    units = [b(inp["lin0_w"]), b(inp["swi1_w1"]), b(inp["swi1_w2"])]
    w1, w3, w2 = (np.asarray(inp["exp_w1"], np.float32),
                  np.asarray(inp["exp_w3"], np.float32),
                  np.asarray(inp["exp_w2"], np.float32))
    for e in range(E):
        units += [b(w1[e]), b(w3[e]), b(w2[e])]
    sh1 = np.asarray(inp["sh_w1"], np.float32)
    sh3 = np.asarray(inp["sh_w3"], np.float32)
    sh2 = np.asarray(inp["sh_w2"], np.float32)
    units += [b(sh1[:, :1024]), b(sh1[:, 1024:]),
              b(sh3[:, :1024]), b(sh3[:, 1024:]),
              b(sh2[:1024, :]), b(sh2[1024:, :]),
              b(inp["lin1_w"]), b(inp["swi2_w1"]), b(inp["swi2_w2"])]
    assert len(units) == N_UNITS
    return np.ascontiguousarray(np.concatenate(units, axis=0))


def _pack_biases(inp):
    cols = []
    for name in ["lin0_b", "swi1_b1", "swi1_b2", "lin1_b", "swi2_b1", "swi2_b2"]:
        v = np.asarray(inp[name], np.float32).reshape(KT, 128).T  # [128, KT]
        cols.append(v)
    return np.ascontiguousarray(np.concatenate(cols, axis=1))  # [128, 6*KT]


def kernel(**inputs):
    global _prog
    from concourse.bass_utils import run_bass_kernel_spmd

    if _prog is None:
        _prog = _build_program()
    nc = _prog

    wall = _pack_weights(inputs)
    biases = _pack_biases(inputs)
    gateT = np.ascontiguousarray(
        np.asarray(inputs["gate_w"], np.float32).T).astype(bf16)
    ident = np.eye(128, dtype=np.float32)
    sel = np.zeros((E, E * 128), dtype=np.float32)
    for e in range(E):
        sel[e, e * 128:(e + 1) * 128] = 1.0

    x = np.asarray(inputs["x"], np.float32).reshape(T, D)
    in_maps = []
    for c in range(NCORES):
        xT = np.ascontiguousarray(x[c * TPC:(c + 1) * TPC, :].T).astype(bf16)
        in_maps.append({
            "wall": wall, "xT": xT, "gateT": gateT, "biases": biases,
            "ident": ident, "sel": sel,
        })

    res = run_bass_kernel_spmd(nc, in_maps, list(range(NCORES)))
    global last_results
    last_results = res
    outT = np.concatenate([res.results[c]["outT"] for c in range(NCORES)], axis=1)
    return np.ascontiguousarray(outT.T).reshape(B, S, D).astype(np.float32)

